# revision 1
# baseline (speedup 1.0000x reference)
"""Multi-head causal attention (B=4, T=2048, C=1024, H=16, DH=64) on 8 trn2 cores.

Sharding: core c owns batch b = c//2 and heads [8*(c%2), 8*(c%2)+8)  (DP over B x TP over H).

Per-core device kernel (all matmuls bf16, fp32 accumulate):
  phase 1: q^T/k^T projections (col-tiled pairs, M=64x2) and v (natural layout,
           heads packed in N=512), from a resident x^T [C, T] in SBUF.
  phase 2: per head-pair, per 512-wide q-chunk, loop causal k-tiles:
           S^T = k q^T (row-tiled pair, K=64x2) -> exp on ACT (scale=1/8) -> bf16 P^T
           -> diag tril mask -> out^T[65, 512] += v_aug.T @ P^T  (row 64 = rowsum
           via ones column in v_aug).
  phase 3: batch-reciprocal of rowsums (PE-transpose to partition-major),
           broadcast, multiply, write out^T fp32.
Host: transposes x / reshapes weights into SBUF-ready layouts (bf16), and
transposes per-head out^T back into [B, T, H*DH].
"""

import numpy as np
import ml_dtypes

B, T, C, H, DH = 4, 2048, 1024, 16, 64
NCORES = 8
HPC = H // 2  # 8 heads per core
CT = C // 128  # 8 contraction tiles
TC = T // 512  # 4 q-chunks
TK = T // 128  # 16 k-tiles

_cache = {}


def build_program(loop_n=1, phases="123"):
    import concourse.bass as bass
    import concourse.bacc as bacc
    import concourse.mybir as mybir
    import concourse.tile as tile
    from concourse.masks import make_upper_triangular, make_identity
    from contextlib import ExitStack

    f32 = mybir.dt.float32
    bf16 = mybir.dt.bfloat16
    EXP = mybir.ActivationFunctionType.Exp

    nc = bacc.Bacc("TRN2", target_bir_lowering=False, debug=False, num_devices=NCORES)
    xT_d = nc.dram_tensor("xT", [128, CT, T], bf16, kind="ExternalInput")
    wq_d = nc.dram_tensor("wq", [128, CT, HPC, DH], bf16, kind="ExternalInput")
    wk_d = nc.dram_tensor("wk", [128, CT, HPC, DH], bf16, kind="ExternalInput")
    wv_d = nc.dram_tensor("wv", [128, CT, HPC, DH], bf16, kind="ExternalInput")
    out_d = nc.dram_tensor("out", [HPC, T, DH], f32, kind="ExternalOutput")

    with tile.TileContext(nc) as tc, ExitStack() as ctx:
        persist = ctx.enter_context(tc.tile_pool(name="persist", bufs=1))
        ptp = ctx.enter_context(tc.tile_pool(name="ptp", bufs=4))
        normp = ctx.enter_context(tc.tile_pool(name="normp", bufs=3))
        pp = ctx.enter_context(tc.tile_pool(name="pp", bufs=2, space="PSUM"))
        pssp = ctx.enter_context(tc.tile_pool(name="pssp", bufs=2, space="PSUM"))
        pop = ctx.enter_context(tc.tile_pool(name="pop", bufs=2, space="PSUM"))

        # persistent SBUF
        xT = persist.tile([128, CT, T], bf16, tag="xT")
        wq = persist.tile([128, CT, HPC, DH], bf16, tag="wq")
        wk = persist.tile([128, CT, HPC, DH], bf16, tag="wk")
        wv = persist.tile([128, CT, HPC, DH], bf16, tag="wv")
        qT = persist.tile([128, HPC // 2, T], bf16, tag="qT")
        kT = persist.tile([128, HPC // 2, T], bf16, tag="kT")
        vsb = persist.tile([128, TK, HPC, DH + 1], bf16, tag="vsb")
        trilT = persist.tile([128, 128], bf16, tag="trilT")
        ident = persist.tile([128, 128], f32, tag="ident")

        # constants (outside the timing loop)
        make_upper_triangular(nc, trilT[:, :], val=1.0, diag=True)
        make_identity(nc, ident[:, :])
        nc.gpsimd.memset(vsb[:, :, :, :], 1.0)

        def body():
            # input DMAs
            nc.sync.dma_start(xT[:, :, :], xT_d[:, :, :])
            nc.sync.dma_start(wq[:, :, :, :], wq_d[:, :, :, :])
            nc.sync.dma_start(wk[:, :, :, :], wk_d[:, :, :, :])
            nc.sync.dma_start(wv[:, :, :, :], wv_d[:, :, :, :])

            # ---- phase 1: projections ----
            # q^T / k^T: col-tiled head pairs -> psum [128, 512] = pair stacked
            for j in range(HPC // 2 if "1" in phases else 0):
                hA, hB = 2 * j, 2 * j + 1
                for c in range(TC):
                    for dst, w in ((qT, wq), (kT, wk)):
                        ps = pp.tile([128, 512], f32, tag="pp")
                        for ct in range(CT):
                            nc.tensor.matmul(
                                ps[0:64, :], w[:, ct, hA, :],
                                xT[:, ct, bass.ts(c, 512)],
                                start=(ct == 0), stop=(ct == CT - 1),
                                tile_position=(0, 0),
                            )
                            nc.tensor.matmul(
                                ps[64:128, :], w[:, ct, hB, :],
                                xT[:, ct, bass.ts(c, 512)],
                                start=(ct == 0), stop=(ct == CT - 1),
                                tile_position=(0, 64),
                            )
                        nc.vector.tensor_copy(dst[:, j, bass.ts(c, 512)], ps[:, :])
            # v natural: [t, (h d)] tiles; heads side by side in N=512
            for tt in range(TK if "1" in phases else 0):
                ps = pp.tile([128, 512], f32, tag="pp")
                for ct in range(CT):
                    nc.tensor.matmul(
                        ps[:, :], xT[:, ct, bass.ts(tt, 128)], wv[:, ct, :, :],
                        start=(ct == 0), stop=(ct == CT - 1),
                    )
                nc.vector.tensor_copy(
                    vsb[:, tt, :, 0:DH],
                    ps[:, :].rearrange("p (h d) -> p h d", h=HPC),
                )

            if "2" not in phases:
                # DCE-proof consumer: write a sliver of the projections out
                nc.gpsimd.dma_start(out_d[0, 0:8, :].rearrange("a b -> (a b)"),
                                    qT[0:1, 0, 0:512])
                nc.gpsimd.dma_start(out_d[1, 0:8, :].rearrange("a b -> (a b)"),
                                    kT[0:1, 0, 0:512])
                nc.gpsimd.dma_start(out_d[2, 0:8, :].rearrange("a b -> (a b)"),
                                    vsb[0:1, 0, :, :].rearrange("p h d -> (p h d)")[0:512])
                return
            # ---- phase 2: attention ----
            for j in range(HPC // 2):
                hA, hB = 2 * j, 2 * j + 1
                for c in range(TC):
                    nr = 4 * c + 4
                    poA = pop.tile([128, 512], f32, tag="po")
                    poB = pop.tile([128, 512], f32, tag="po")
                    pss = {}
                    pts = {}

                    def emit_S(r):
                        ps = pssp.tile([128, 1024], f32, tag="pss")
                        pss[r] = ps
                        nc.tensor.matmul(
                            ps[:, 0:512], kT[0:64, j, bass.ts(r, 128)],
                            qT[0:64, j, bass.ts(c, 512)],
                            start=True, stop=True, tile_position=(0, 0),
                        )
                        nc.tensor.matmul(
                            ps[:, 512:1024], kT[64:128, j, bass.ts(r, 128)],
                            qT[64:128, j, bass.ts(c, 512)],
                            start=True, stop=True, tile_position=(64, 0),
                        )
                        pt = ptp.tile([128, 1024], bf16, tag="pt")
                        pts[r] = pt
                        nc.scalar.activation(pt[:, :], ps[:, :], EXP, scale=0.125)
                        if "x" in phases:  # diagnostic: double ACT load
                            nc.scalar.activation(pt[:, :], ps[:, :], EXP, scale=0.125)
                        if r >= 4 * c:  # diagonal tile: tril mask
                            jd = r - 4 * c
                            nc.vector.tensor_mul(
                                pt[:, bass.ts(jd, 128)], pt[:, bass.ts(jd, 128)],
                                trilT[:, :])
                            nc.vector.tensor_mul(
                                pt[:, 512 + jd * 128:512 + (jd + 1) * 128],
                                pt[:, 512 + jd * 128:512 + (jd + 1) * 128],
                                trilT[:, :])

                    def emit_PV(r):
                        vlo = max(0, (r - 4 * c)) * 128
                        pt = pts.pop(r)
                        pss.pop(r)
                        nc.tensor.matmul(
                            poA[0:DH + 1, vlo:512], vsb[:, r, hA, :],
                            pt[:, vlo:512],
                            start=(r == 0), stop=(r == nr - 1),
                            skip_group_check=True,
                        )
                        nc.tensor.matmul(
                            poB[0:DH + 1, vlo:512], vsb[:, r, hB, :],
                            pt[:, 512 + vlo:1024],
                            start=(r == 0), stop=(r == nr - 1),
                            skip_group_check=True,
                        )

                    for rr_i in range(0, nr, 2):
                        emit_S(rr_i)
                        emit_S(rr_i + 1)
                        if rr_i >= 2:
                            emit_PV(rr_i - 2)
                            emit_PV(rr_i - 1)
                    emit_PV(nr - 2)
                    emit_PV(nr - 1)

                    # normalize + write natural-layout output rows for this chunk
                    for h, po in ((hA, poA), (hB, poB)):
                        ou_s = normp.tile([DH + 1, 512], f32, tag="ou_s")
                        nc.vector.tensor_copy(ou_s[:, :], po[0:DH + 1, :])
                        pt_o = pp.tile([128, 4 * DH], f32, tag="pp")
                        prs = pp.tile([128, 4], f32, tag="pp")
                        for t4 in range(4):
                            nc.tensor.transpose(
                                pt_o[:, bass.ts(t4, DH)],
                                ou_s[0:DH, bass.ts(t4, 128)], ident[0:DH, 0:DH])
                            nc.tensor.transpose(
                                prs[:, t4:t4 + 1],
                                ou_s[DH:DH + 1, bass.ts(t4, 128)],
                                ident[DH:DH + 1, DH:DH + 1])
                        rsc = normp.tile([128, 4], f32, tag="rsc")
                        nc.vector.reciprocal(rsc[:, :], prs[:, :])
                        on_t = normp.tile([128, 4, DH], f32, tag="on_t")
                        for t4 in range(4):
                            nc.vector.tensor_scalar_mul(
                                on_t[:, t4, :], pt_o[:, bass.ts(t4, DH)],
                                rsc[:, t4:t4 + 1])
                        nc.sync.dma_start(
                            out_d[h, bass.ts(c, 512), :].rearrange(
                                "(t p) d -> p t d", p=128),
                            on_t[:, :, :])


        if loop_n > 1:
            with tc.For_i(0, loop_n, 1):
                body()
        else:
            body()

    nc.compile()
    return nc


def _prep_core_inputs(x, Wq, Wk, Wv, core):
    bf = ml_dtypes.bfloat16
    b = core // 2
    hs = (core % 2) * HPC
    # x^T in SBUF layout [p, ct, t]
    xT = np.ascontiguousarray(x[b].T).astype(bf)          # [C, T]
    xT = xT.reshape(CT, 128, T).transpose(1, 0, 2)        # [128, CT, T]
    ws = []
    for W in (Wq, Wk, Wv):
        w = W[hs:hs + HPC].transpose(1, 0, 2).astype(bf)  # [C, HPC, DH]
        w = w.reshape(CT, 128, HPC, DH).transpose(1, 0, 2, 3)  # [128, CT, HPC, DH]
        ws.append(np.ascontiguousarray(w))
    return {
        "xT": np.ascontiguousarray(xT),
        "wq": ws[0], "wk": ws[1], "wv": ws[2],
    }


def run_on_device(inputs, loop_n=1, trace=False, phases="123"):
    """Build (cached), run on 8 cores, return list of per-core {'out': [HPC, DH, T]}."""
    from concourse.bass_utils import run_bass_kernel_spmd

    key = (loop_n, phases)
    if key not in _cache:
        _cache[key] = build_program(loop_n, phases)
    nc = _cache[key]
    in_maps = [
        _prep_core_inputs(inputs["x"], inputs["Wq"], inputs["Wk"], inputs["Wv"], c)
        for c in range(NCORES)
    ]
    res = run_bass_kernel_spmd(nc, in_maps, list(range(NCORES)), trace=trace)
    return res


def kernel(x, Wq, Wk, Wv):
    res = run_on_device({"x": x, "Wq": Wq, "Wk": Wk, "Wv": Wv})
    out = np.empty((B, T, H * DH), np.float32)
    for core in range(NCORES):
        b = core // 2
        hs = (core % 2) * HPC
        o = res.results[core]["out"]  # [HPC, T, DH]
        out[b, :, hs * DH:(hs + HPC) * DH] = o.transpose(1, 0, 2).reshape(T, HPC * DH)
    return out

